# revision 12
# baseline (speedup 1.0000x reference)
"""Cached multi-head attention, sharded over heads across 8 TRN2 NeuronCores.

Per-core work (2 of 16 heads, all 8 batches):
  qkv^T via Wqkv-stationary matmuls producing [outdim, token] layout;
  per (batch, head): scores^T[k, q] with k^T-stationary matmuls where the
  cached keys are fp8-e3m4 (PE takes mixed fp8 stationary x fp16 moving)
  and the new-key block is the fp16 k^T slab straight out of qkvT; exp on
  ACT in 512-key groups software-pipelined S0 S1 A0 S2 A1 ... so the
  scalar engine has 2x slack; causal mask on the last 128-key block;
  attn@V with e3m4 v-stationary accumulating out^T[d, q]; softmax
  denominator folded incrementally on DVE as exp groups land, finished by
  gpsimd partition_all_reduce + reciprocal_approx_fast (no PE); output
  projection with attn-stationary / Wproj-moving matmuls emitting natural
  [token, 2048] partials per batch; the host sums the 8 partials and adds
  the bias.

Layout notes:
  - k cache is host-transposed to [h, b, HD, CACHE] fp8-e3m4 (3968B lines).
  - v cache is oct-packed fp8-e3m4 ([4, 128, 1024] per (h,b): eight
    128-key chunks side by side in the free dim) so DMA runs are 1KB.
  - DMA descriptor generation is engine-hosted, so streams are spread:
    cache on sync, Wqkv on gpsimd, x/Wproj on scalar, outputs on vector.
  - startup DMAs are issued before const setup; wq arrives per 128-col
    group (q_h0 first) so the first QKV matmul starts after ~1.6 MB.
"""

import numpy as np
import ml_dtypes

import concourse.bacc as bacc
import concourse.mybir as mybir
import concourse.tile as tile
from concourse import bass_isa
from concourse.bass_utils import run_bass_kernel_spmd

B, Q, D = 8, 128, 2048
H, HD = 16, 128
CACHE = 3968
K = CACHE + Q          # 4096
NCORES = 8
HPC = H // NCORES      # heads per core
NKC = K // 128         # 32 key chunks (chunk 31 = the new block)
NOCT = NKC // 8        # 4 oct-packed v groups
TOK = B * Q            # 1024 tokens
QKV_COLS = 3 * HPC * HD  # 768 per core
SCALE = 1.0 / float(np.sqrt(HD))

F16 = mybir.dt.float16
F32 = mybir.dt.float32
F8E3 = mybir.dt.float8e3

PAIRS = [(b, hh) for b in range(B) for hh in range(HPC)]
PREFETCH = 4           # cache-pair DMA prefetch distance

_STATE = {}


def build_nc(reps=1):
    nc = bacc.Bacc("TRN2", target_bir_lowering=False, debug=False)

    xt_d = nc.dram_tensor("xt", [D, TOK], F16, kind="ExternalInput")
    wq_d = nc.dram_tensor("wqkv", [D, QKV_COLS], F16, kind="ExternalInput")
    kt_d = nc.dram_tensor("kt", [HPC, B, HD, CACHE], F8E3, kind="ExternalInput")
    vp_d = nc.dram_tensor("vp", [HPC, B, NOCT, 128, 1024], F8E3, kind="ExternalInput")
    wp_d = nc.dram_tensor("wp", [HPC * HD, D], F16, kind="ExternalInput")
    out_d = nc.dram_tensor("out", [TOK, D], F16, kind="ExternalOutput")

    with tile.TileContext(nc) as tc:
        with (
            tc.tile_pool(name="const", bufs=1) as cpool,
            tc.tile_pool(name="xw", bufs=1) as xwpool,
            tc.tile_pool(name="qkv", bufs=1) as qkvpool,
            tc.tile_pool(name="vnew", bufs=1) as vnewpool,
            tc.tile_pool(name="attn", bufs=1) as attnpool,
            tc.tile_pool(name="kt", bufs=5) as ktpool,
            tc.tile_pool(name="v", bufs=5) as vpool,
            tc.tile_pool(name="p", bufs=2) as ppool,
            tc.tile_pool(name="fold", bufs=2) as foldpool,
            tc.tile_pool(name="small", bufs=2) as smallpool,
            tc.tile_pool(name="ostage", bufs=2) as opool,
            tc.tile_pool(name="ps_q", bufs=2, space="PSUM") as psum_q,
            tc.tile_pool(name="ps_s", bufs=2, space="PSUM") as psum_s,
            tc.tile_pool(name="ps_o", bufs=2, space="PSUM") as psum_o,
        ):
            xt_r = xt_d.ap().rearrange("(t p) n -> p t n", p=128)
            wq_r = wq_d.ap().rearrange("(t p) c -> p t c", p=128)
            wp_r = wp_d.ap().rearrange("(t p) c -> p t c", p=128)
            out_r = out_d.ap().rearrange("(b p) c -> b p c", p=128)

            for _rep in range(reps):
                # ---- startup DMA issues (before everything else) ----
                wq_sb = xwpool.tile([128, D // 128, QKV_COLS], F16,
                                    tag="wq", name="wq_sb")
                xt_half = [
                    xwpool.tile([128, D // 128, 512], F16, tag="xt0",
                                name="xt0"),
                    xwpool.tile([128, D // 128, 512], F16, tag="xt1",
                                name="xt1"),
                ]

                def load_xt_piece(t, half):
                    dsl = slice(half * 8, (half + 1) * 8)
                    nc.scalar.dma_start(
                        xt_half[t][:, dsl, :],
                        xt_r[:, dsl, t * 512:(t + 1) * 512],
                    )

                def load_wq_oc(oc):
                    nc.gpsimd.dma_start(
                        wq_sb[:, :, oc * 128:(oc + 1) * 128],
                        wq_r[:, :, oc * 128:(oc + 1) * 128],
                    )

                pair_bufs = {}

                def emit_pair_dma(i):
                    if i >= len(PAIRS):
                        return
                    # pairs 0-4 ride the gpsimd queue BEHIND the startup-
                    # critical wq pieces (no HBM steal from the QKV path);
                    # pairs 5+ ride sync, auto-gated by the kt/v pool
                    # buffer semaphores (bufs=5) until attention consumes.
                    eng = nc.gpsimd if i <= 4 else nc.sync
                    b, hh = PAIRS[i]
                    kt_sb = ktpool.tile([128, CACHE], F8E3, tag="kt",
                                        name=f"kt{i}")
                    eng.dma_start(kt_sb[:], kt_d.ap()[hh, b])
                    v_sb = vpool.tile([128, NOCT, 1024], F8E3, tag="v",
                                      name=f"v{i}")
                    eng.dma_start(
                        v_sb[:], vp_d.ap()[hh, b].rearrange("q k d -> k q d")
                    )
                    pair_bufs[i] = (kt_sb, v_sb)

                load_xt_piece(0, 0)
                load_xt_piece(0, 1)
                load_wq_oc(0)
                load_wq_oc(2)
                load_wq_oc(4)
                emit_pair_dma(0)
                load_wq_oc(1)
                load_wq_oc(3)
                load_wq_oc(5)
                emit_pair_dma(1)
                emit_pair_dma(2)
                emit_pair_dma(3)
                load_xt_piece(1, 0)
                load_xt_piece(1, 1)
                wp_sb = xwpool.tile([128, HPC, D], F16, tag="wp", name="wp_sb")
                nc.scalar.dma_start(wp_sb[:], wp_r)

                # ---- constants ----
                ones_full = cpool.tile([128, 128], F16, tag="ones",
                                       name="ones_full")
                nc.vector.memset(ones_full[:], 1.0)
                zeros_full = cpool.tile([128, 128], F32, tag="zeros",
                                        name="zeros_full")
                nc.vector.memset(zeros_full[:], 0.0)
                # causal mask for the last key block: keep (p=key j',
                # free=query i) where i >= j'  ->  base + ch*(-1) + x*1 >= 0
                maskneg = cpool.tile([128, 128], F32, tag="mask",
                                     name="maskneg")
                nc.gpsimd.affine_select(
                    maskneg[:], zeros_full[:], pattern=[[1, 128]],
                    compare_op=mybir.AluOpType.is_ge, fill=-1e30,
                    base=0, channel_multiplier=-1,
                )
                ident = cpool.tile([128, 128], F16, tag="ident", name="ident")
                nc.gpsimd.affine_select(
                    ident[:], ones_full[:], pattern=[[1, 128]],
                    compare_op=mybir.AluOpType.is_equal, fill=0.0,
                    base=0, channel_multiplier=-1,
                )

                qkvT = [
                    qkvpool.tile([128, QKV_COLS // 128, 512], F16,
                                 tag=f"qkvT{t}", name=f"qkvT{t}")
                    for t in range(2)
                ]
                vnew_sb = vnewpool.tile([128, HPC, B, HD], F16,
                                        tag="vnew", name="vnew_sb")
                attn_sb = attnpool.tile([128, HPC, TOK], F16,
                                        tag="attn", name="attn_sb")

                def qkv_group(t, oc):
                    ps = psum_q.tile([128, 512], F32, tag="q", name="ps_qkv")
                    for dt_ in range(D // 128):
                        nc.tensor.matmul(
                            ps[:],
                            wq_sb[:, dt_, oc * 128:(oc + 1) * 128],
                            xt_half[t][:, dt_, :],
                            start=(dt_ == 0), stop=(dt_ == D // 128 - 1),
                        )
                    nc.vector.tensor_copy(qkvT[t][:, oc, :], ps[:])

                def vnew_transposes(t, hh):
                    # rides the scores-psum ring (F16 [128,128] fits the
                    # 2KB slot); transposes are rare so the serialization
                    # with score groups is harmless.
                    for bb in range(4):
                        b = 4 * t + bb
                        ps_t = psum_s.tile([128, 128], F16, tag="s",
                                           name="ps_tr")
                        nc.tensor.transpose(
                            ps_t[:],
                            qkvT[t][:, 2 * HPC + hh, bb * 128:(bb + 1) * 128],
                            ident[:],
                        )
                        nc.scalar.copy(vnew_sb[:, hh, b, :], ps_t[:])

                def qkv_head(t, hh):
                    for oc in (hh, HPC + hh, 2 * HPC + hh):
                        qkv_group(t, oc)
                    vnew_transposes(t, hh)

                # deferred-finish state: pair i's last AV group + normalize
                # run inside pair i+1's score phase, when its exp is long
                # done — removes the exp3->A3 latency stall at pair ends.
                prev = {}

                def av_group(st, g):
                    b, hh = st["bh"]
                    for j in range(8):
                        c = 8 * g + j
                        if c < NKC - 1:
                            oc8, jj = divmod(c, 8)
                            stat = st["v"][:, oc8, jj * 128:(jj + 1) * 128]
                        else:
                            stat = vnew_sb[:, hh, b, :]
                        nc.tensor.matmul(
                            st["ps_o"][:], stat,
                            st["pT"][:, c * 128:(c + 1) * 128],
                            start=(c == 0), stop=(c == NKC - 1),
                        )

                def finish_prev():
                    if not prev:
                        return
                    b, hh = prev["bh"]
                    av_group(prev, 3)
                    nc.vector.tensor_mul(
                        attn_sb[:, hh, b * 128:(b + 1) * 128],
                        prev["ps_o"][:], prev["inv"][:]
                    )
                    prev.clear()

                def attention_pair(i):
                    b, hh = PAIRS[i]
                    t, bb = divmod(b, 4)
                    kt_sb, v_sb = pair_bufs.pop(i)
                    qT = qkvT[t][:, hh, bb * 128:(bb + 1) * 128]
                    knew = qkvT[t][:, HPC + hh, bb * 128:(bb + 1) * 128]
                    pT = ppool.tile([128, K], F16, tag="pT", name=f"pT{i}")
                    ps_o = psum_o.tile([128, 128], F32, tag="o", name="ps_av")
                    fold = foldpool.tile([128, 1024], F16, tag="fold",
                                         name=f"fold{i}")
                    st = {"bh": (b, hh), "v": v_sb, "pT": pT, "ps_o": ps_o}

                    def score_group(g):
                        ps = psum_s.tile([128, 1024], F32, tag="s",
                                         name=f"ps_s{g}")
                        for j in range(8):
                            kc = 8 * g + j
                            stat = (kt_sb[:, kc * 128:(kc + 1) * 128]
                                    if kc < NKC - 1 else knew)
                            nc.tensor.matmul(
                                ps[:, j * 128:(j + 1) * 128], stat, qT,
                                start=True, stop=True,
                            )
                        if g == 3:
                            nc.vector.tensor_add(
                                ps[:, 896:1024], ps[:, 896:1024], maskneg[:]
                            )
                        slab = pT[:, g * 1024:(g + 1) * 1024]
                        nc.scalar.activation(
                            slab, ps[:],
                            mybir.ActivationFunctionType.Exp, scale=SCALE,
                        )
                        # incremental denominator fold (DVE) as slabs land
                        if g == 0:
                            nc.vector.tensor_copy(fold[:], slab)
                        else:
                            nc.vector.tensor_add(fold[:], fold[:], slab)

                    # S0 S1 [A3 of prev pair] S2 A0 S3 A1 A2
                    score_group(0)
                    score_group(1)
                    finish_prev()
                    score_group(2)
                    av_group(st, 0)
                    score_group(3)
                    av_group(st, 1)
                    av_group(st, 2)

                    # denominator tail: 1024 -> 128, all-reduce, reciprocal
                    for w in (512, 256, 128):
                        nc.vector.tensor_add(
                            fold[:, 0:w], fold[:, 0:w], fold[:, w:2 * w]
                        )
                    allr = smallpool.tile([128, 128], F32, tag="ar",
                                          name=f"ar{i}")
                    nc.gpsimd.partition_all_reduce(
                        allr[:], fold[:, 0:128], channels=128,
                        reduce_op=bass_isa.ReduceOp.add,
                    )
                    inv = smallpool.tile([128, 128], F32, tag="inv",
                                         name=f"inv{i}")
                    nc.vector.reciprocal_approx_fast(inv[:], allr[:])
                    st["inv"] = inv
                    prev.update(st)

                def proj_batch(b):
                    o_sb = opool.tile([128, D], F16, tag="o", name=f"o{b}")
                    for cb in range(4):
                        ps = psum_q.tile([128, 512], F32, tag="q",
                                         name="ps_proj")
                        for ht in range(HPC):
                            nc.tensor.matmul(
                                ps[:],
                                attn_sb[:, ht, b * 128:(b + 1) * 128],
                                wp_sb[:, ht, cb * 512:(cb + 1) * 512],
                                start=(ht == 0), stop=(ht == HPC - 1),
                            )
                        nc.vector.tensor_copy(
                            o_sb[:, cb * 512:(cb + 1) * 512], ps[:]
                        )
                    nc.gpsimd.dma_start(out_r[b], o_sb[:])

                # ---- schedule ----
                qkv_head(0, 0)
                for i in range(len(PAIRS)):
                    emit_pair_dma(i + PREFETCH)
                    if i == 1:
                        qkv_head(0, 1)
                    elif i == 3:
                        proj_batch(0)
                    elif i == 4:
                        qkv_head(1, 0)
                    elif i == 5:
                        proj_batch(1)
                    elif i == 6:
                        qkv_head(1, 1)
                    elif i == 7:
                        proj_batch(2)
                    elif i == 9:
                        proj_batch(3)
                    elif i == 11:
                        proj_batch(4)
                    elif i == 13:
                        proj_batch(5)
                    elif i == 15:
                        proj_batch(6)
                    attention_pair(i)
                finish_prev()
                proj_batch(7)

    nc.compile()
    return nc


def prepare_in_maps(x, k_cache, v_cache, Wqkv, Wproj):
    xt = np.ascontiguousarray(x.reshape(TOK, D).T, dtype=np.float16)
    in_maps = []
    for c in range(NCORES):
        h0 = c * HPC
        cols = []
        for i3 in range(3):
            for hh in range(HPC):
                h = h0 + hh
                cols.append(Wqkv[:, i3 * D + h * HD:(i3 * D + (h + 1) * HD)])
        wq = np.ascontiguousarray(np.concatenate(cols, axis=1), dtype=np.float16)
        ks = k_cache[:, h0:h0 + HPC]                  # [B, HPC, CACHE, HD]
        kt = np.ascontiguousarray(
            np.transpose(ks, (1, 0, 3, 2))
        ).astype(ml_dtypes.float8_e3m4)               # [HPC, B, HD, CACHE]
        vs = v_cache[:, h0:h0 + HPC]                  # [B, HPC, CACHE, HD]
        # oct-pack v: [HPC, B, NOCT, 128, 1024]; oct q holds chunks
        # 8q..8q+7 as [key-in-chunk, d] side by side; chunk 31 slot unused.
        vp = np.zeros((HPC, B, NOCT, 128, 1024), ml_dtypes.float8_e3m4)
        nfull = CACHE // 1024                          # 3 full octs
        full = vs[:, :, :nfull * 1024, :].reshape(B, HPC, nfull, 8, 128, HD)
        vp[:, :, :nfull] = (
            np.transpose(full, (1, 0, 2, 4, 3, 5))
            .reshape(HPC, B, nfull, 128, 1024)
            .astype(ml_dtypes.float8_e3m4)
        )
        ntail = (CACHE - nfull * 1024) // 128          # 7 tail chunks
        tail = vs[:, :, nfull * 1024:, :].reshape(B, HPC, ntail, 128, HD)
        vp[:, :, nfull, :, 0:ntail * 128] = (
            np.transpose(tail, (1, 0, 3, 2, 4))
            .reshape(HPC, B, 128, ntail * 128)
            .astype(ml_dtypes.float8_e3m4)
        )
        wp = np.ascontiguousarray(
            Wproj[h0 * HD:(h0 + HPC) * HD, :], dtype=np.float16
        )
        in_maps.append({"xt": xt, "wqkv": wq, "kt": kt, "vp": vp, "wp": wp})
    return in_maps


def postprocess(results, bproj):
    total = np.zeros((TOK, D), dtype=np.float32)
    for c in range(NCORES):
        total += results[c]["out"].astype(np.float32)
    out = total + bproj.astype(np.float32)[None, :]
    return np.ascontiguousarray(out.reshape(B, Q, D), dtype=np.float32)


def kernel(x, k_cache, v_cache, Wqkv, Wproj, bproj):
    if "nc" not in _STATE:
        _STATE["nc"] = build_nc()
    nc = _STATE["nc"]
    in_maps = prepare_in_maps(
        np.asarray(x), np.asarray(k_cache), np.asarray(v_cache),
        np.asarray(Wqkv), np.asarray(Wproj)
    )
    res = run_bass_kernel_spmd(nc, in_maps, list(range(NCORES)))
    return postprocess(res.results, np.asarray(bproj))
